# revision 1
# baseline (speedup 1.0000x reference)

"""Causal attention (no head split) on 8 trn2 NeuronCores.

Reference computation (per batch b):
    q = x @ Wq^T ; k = x @ Wk^T ; v = x @ Wv^T          (nn.Linear convention)
    wei = softmax(mask(q @ k^T / sqrt(C)))               (causal)
    out = wei @ v

Sharding: 2 cores per batch (B=4). Within a batch, queries are split into
eight 256-row strips; core role A takes strips {0,2,4,6} (rows [512j,512j+256)),
role B takes {1,3,5,7}. Every core runs the IDENTICAL instruction stream
(single SPMD NEFF); role differences are carried entirely by input data
(which query columns are fed, and causal-mask tiles).

On-device layout (everything fp32r = e8m11, PE-native fast fp32):
    xT   [C, T]  : x^T for the batch        (host pre-transposed + rounded)
    xqT  [C, 1024]: x^T columns of this core's 4 query strips
    w*T  [C, C]  : weights transposed to [c, d]
Per query strip j (256 queries) and kv chunk c (256 keys), c <= 2j+1:
    S^T[s, t] = K^T(lhsT) @ Q^T(rhs) accumulated over d -> PSUM
    P^T = exp(S^T / 32) * mask            (ACT exp, DVE mask-mul)
    rowsum[t] += ones^T @ P^T             (1-row matmul)
    O^T[d, t] += V(lhsT) @ P^T(rhs)       (accumulated in SBUF via DVE adds)
Final softmax normalization (divide by rowsum) happens on the host.
"""
import os
import numpy as np

import concourse.bass as bass
from concourse import bacc
import concourse.mybir as mybir
from concourse.tile import TileContext
from concourse import bass_utils

B, T, C = 4, 2048, 1024
P = 128
CS = C // P          # 8 contraction subtiles
NCH = T // 256       # 8 kv chunks of 256
QS = 4               # query strips per core
SW = 256             # strip width
SCALE = 1.0 / np.sqrt(C)  # 1/32

F32R = mybir.dt.float32r
F32 = mybir.dt.float32


def round_fp32r(x: np.ndarray) -> np.ndarray:
    """Round fp32 to fp32r (e8m11): round-to-nearest-even to 11 mantissa bits."""
    x = np.ascontiguousarray(x, dtype=np.float32)
    bits = x.view(np.uint32)
    lsb = (bits >> 12) & 1
    out = (bits + 0x7FF + lsb) & np.uint32(0xFFFFF000)
    return out.view(np.float32)


def build():
    nc = bacc.Bacc(trn_type="TRN2", name="causal_attn")
    xT = nc.dram_tensor("xT", [C, T], F32R, kind="ExternalInput")
    xqT = nc.dram_tensor("xqT", [C, QS * SW], F32R, kind="ExternalInput")
    wqT = nc.dram_tensor("wqT", [C, C], F32R, kind="ExternalInput")
    wkT = nc.dram_tensor("wkT", [C, C], F32R, kind="ExternalInput")
    wvT = nc.dram_tensor("wvT", [C, C], F32R, kind="ExternalInput")
    masks = nc.dram_tensor("masks", [P, 4, SW], F32R, kind="ExternalInput")
    ones = nc.dram_tensor("ones", [P, 1], F32R, kind="ExternalInput")
    outT = nc.dram_tensor("outT", [C, QS * SW], F32, kind="ExternalOutput")
    rows = nc.dram_tensor("rows", [1, QS * SW], F32, kind="ExternalOutput")

    xT_r = xT.rearrange("(cs p) t -> p cs t", p=P)
    xqT_r = xqT.rearrange("(cs p) t -> p cs t", p=P)
    wqT_r = wqT.rearrange("(cs p) d -> p cs d", p=P)
    wkT_r = wkT.rearrange("(cs p) d -> p cs d", p=P)
    wvT_r = wvT.rearrange("(cs p) d -> p cs d", p=P)
    outT_r = outT.rearrange("(ds p) t -> p ds t", p=P)
    rows_r = rows.rearrange("p (a b) -> p a b", a=QS)

    with TileContext(nc) as tc:
        with tc.tile_pool(name="keep", bufs=1) as keep, \
             tc.tile_pool(name="wpool", bufs=2) as wpool, \
             tc.tile_pool(name="stream", bufs=2) as stream, \
             tc.tile_pool(name="ppool", bufs=3) as ppool, \
             tc.tile_pool(name="psA", bufs=2, space="PSUM") as psA, \
             tc.tile_pool(name="psS", bufs=2, space="PSUM") as psS, \
             tc.tile_pool(name="psO", bufs=2, space="PSUM") as psO, \
             tc.tile_pool(name="psR", bufs=2, space="PSUM") as psR:

            qT = keep.tile([P, CS, QS * SW], F32R, tag="qT")       # 32KB/part
            oT = keep.tile([P, CS, QS * SW], F32, tag="oT")        # 32KB/part
            msk = keep.tile([P, 4, SW], F32R, tag="msk")
            ones_t = keep.tile([P, 1], F32R, tag="ones")
            rowsum = keep.tile([1, QS, SW], F32, tag="rowsum")
            nc.sync.dma_start(msk[:], masks[:])
            nc.sync.dma_start(ones_t[:], ones[:])

            # ---- Phase Q: Q^T for the 4 query strips ----
            wq = wpool.tile([P, CS, C], F32R, tag="w")
            nc.sync.dma_start(wq[:], wqT_r[:])
            for j in range(QS):
                xq = stream.tile([P, CS, SW], F32R, tag="xt")
                nc.sync.dma_start(xq[:], xqT_r[:, :, j * SW:(j + 1) * SW])
                for ds in range(CS):
                    pq = psA.tile([P, SW], F32, tag="prod")
                    for cs in range(CS):
                        nc.tensor.matmul(
                            pq[:], wq[:, cs, ds * P:(ds + 1) * P], xq[:, cs],
                            start=(cs == 0), stop=(cs == CS - 1))
                    nc.scalar.copy(qT[:, ds, j * SW:(j + 1) * SW], pq[:])

            # ---- K/V weights (wk evicts into the second w slot; wv reuses wq's) ----
            wk = wpool.tile([P, CS, C], F32R, tag="w")
            nc.sync.dma_start(wk[:], wkT_r[:])
            wv = wpool.tile([P, CS, C], F32R, tag="w")
            nc.sync.dma_start(wv[:], wvT_r[:])

            # ---- Chunk loop: produce K^T/V for chunk c, then attend all strips ----
            for c in range(NCH):
                xt = stream.tile([P, CS, 256], F32R, tag="xt")
                nc.sync.dma_start(xt[:], xT_r[:, :, c * 256:(c + 1) * 256])

                kTc = stream.tile([P, CS, 256], F32R, tag="kt")
                for ds in range(CS):
                    pk = psA.tile([P, 256], F32, tag="prod")
                    for cs in range(CS):
                        nc.tensor.matmul(
                            pk[:], wk[:, cs, ds * P:(ds + 1) * P], xt[:, cs],
                            start=(cs == 0), stop=(cs == CS - 1))
                    nc.scalar.copy(kTc[:, ds], pk[:])

                vc = stream.tile([P, 2, C], F32R, tag="vt")
                for dh in range(2):
                    for ss in range(2):
                        pv = psA.tile([P, 512], F32, tag="prod")
                        for cs in range(CS):
                            nc.tensor.matmul(
                                pv[:], xt[:, cs, ss * P:(ss + 1) * P],
                                wv[:, cs, dh * 512:(dh + 1) * 512],
                                start=(cs == 0), stop=(cs == CS - 1))
                        nc.scalar.copy(vc[:, ss, dh * 512:(dh + 1) * 512], pv[:])

                # strips that attend to chunk c: 2j+1 >= c
                for j in range(QS):
                    if 2 * j + 1 < c:
                        continue
                    tsl = slice(j * SW, (j + 1) * SW)

                    st = psS.tile([P, 2, SW], F32, tag="st")
                    for ss in range(2):
                        for ds in range(CS):
                            nc.tensor.matmul(
                                st[:, ss], kTc[:, ds, ss * P:(ss + 1) * P],
                                qT[:, ds, tsl],
                                start=(ds == 0), stop=(ds == CS - 1))

                    pT = ppool.tile([P, 2, SW], F32R, tag="pT")
                    for ss in range(2):
                        nc.scalar.activation(
                            pT[:, ss], st[:, ss],
                            mybir.ActivationFunctionType.Exp, scale=float(SCALE))

                    midx = None
                    if c == 2 * j:
                        midx = 0
                    elif c == 2 * j + 1:
                        midx = 1
                    if midx is not None:
                        for ss in range(2):
                            nc.vector.tensor_mul(
                                pT[:, ss], pT[:, ss], msk[:, midx * 2 + ss])

                    rw = psR.tile([1, SW], F32, tag="rw")
                    for ss in range(2):
                        nc.tensor.matmul(
                            rw[:], ones_t[:], pT[:, ss],
                            start=(ss == 0), stop=(ss == 1))
                    if c == 0:
                        nc.vector.tensor_copy(rowsum[:, j], rw[:])
                    else:
                        nc.vector.tensor_add(rowsum[:, j], rowsum[:, j], rw[:])

                    for q4 in range(4):   # d quarters: ds pair (2q4, 2q4+1)
                        po = psO.tile([P, 2, SW], F32, tag="po")
                        for i in range(2):
                            ds = 2 * q4 + i
                            for ss in range(2):
                                nc.tensor.matmul(
                                    po[:, i], vc[:, ss, ds * P:(ds + 1) * P],
                                    pT[:, ss],
                                    start=(ss == 0), stop=(ss == 1))
                        osl = oT[:, 2 * q4:2 * q4 + 2, tsl]
                        if c == 0:
                            nc.vector.tensor_copy(osl, po[:])
                        else:
                            nc.vector.tensor_add(osl, osl, po[:])

            # ---- store ----
            nc.sync.dma_start(outT_r[:], oT[:])
            nc.sync.dma_start(rows_r[:], rowsum[:])

    nc.compile()
    return nc


_NC = None


def _get_nc():
    global _NC
    if _NC is None:
        _NC = build()
    return _NC


def make_in_maps(x, Wq, Wk, Wv):
    x = np.asarray(x, dtype=np.float32)
    wqT = round_fp32r(np.asarray(Wq, np.float32).T)
    wkT = round_fp32r(np.asarray(Wk, np.float32).T)
    wvT = round_fp32r(np.asarray(Wv, np.float32).T)
    ones = np.ones((P, 1), np.float32)

    # mask tiles [p, midx*2+ss, t]: tri = 1 if (ss*128+p) <= t
    s_idx = (np.arange(2)[:, None, None] * P + np.arange(P)[None, :, None])  # [ss,p,1]
    tri = (s_idx <= np.arange(SW)[None, None, :]).astype(np.float32)         # [ss,p,t]
    tri = np.ascontiguousarray(tri.transpose(1, 0, 2))                       # [p,ss,t]
    zeros = np.zeros((P, 2, SW), np.float32)
    ones2 = np.ones((P, 2, SW), np.float32)
    mask_A = np.ascontiguousarray(
        np.concatenate([tri, zeros], axis=1), np.float32)                    # [p,4,t]
    mask_B = np.ascontiguousarray(
        np.concatenate([ones2, tri], axis=1), np.float32)

    xT = [round_fp32r(x[b].T) for b in range(B)]
    in_maps = []
    for core in range(8):
        b, role = divmod(core, 2)
        cols = np.concatenate(
            [np.arange(512 * j + SW * role, 512 * j + SW * role + SW)
             for j in range(QS)])
        xqT = np.ascontiguousarray(xT[b][:, cols])
        in_maps.append({
            "xT": xT[b],
            "xqT": xqT,
            "wqT": wqT, "wkT": wkT, "wvT": wvT,
            "masks": mask_A if role == 0 else mask_B,
            "ones": ones,
        })
    return in_maps


def assemble(results):
    out = np.empty((B, T, C), np.float32)
    for core in range(8):
        b, role = divmod(core, 2)
        oT = results[core]["outT"]                   # [C, 1024]
        rsum = results[core]["rows"].reshape(QS * SW)
        o = oT.T / rsum[:, None]
        for j in range(QS):
            r0 = 512 * j + SW * role
            out[b, r0:r0 + SW] = o[j * SW:(j + 1) * SW]
    return out


def kernel(x, Wq, Wk, Wv):
    nc = _get_nc()
    in_maps = make_in_maps(x, Wq, Wk, Wv)
    res = bass_utils.run_bass_kernel_spmd(nc, in_maps, core_ids=list(range(8)))
    return assemble(res.results)


def run_traced(x, Wq, Wk, Wv):
    """Like kernel() but with NTFF tracing; returns (out, BassKernelResults)."""
    import axon_trace_shim  # noqa: F401
    nc = _get_nc()
    in_maps = make_in_maps(x, Wq, Wk, Wv)
    res = bass_utils.run_bass_kernel_spmd(
        nc, in_maps, core_ids=list(range(8)), trace=True,
        trace_cores=list(range(8)))
    return assemble(res.results), res
